# revision 5
# baseline (speedup 1.0000x reference)
"""Causal self-attention Bass/Tile kernel for 8 Trainium2 NeuronCores (v2).

Problem (hardcoded): x (4, 2048, 1024) f32, w_attn (1024, 3072), w_proj
(1024, 1024).  H=16 heads, D=64.  Output: (4, 2048, 1024) f32.

Sharding: core c handles batch b = c // 2 and head-group hg = c % 2
(8 heads each).  Data parallel on B, tensor parallel on heads: each core
gets the w_attn columns for its heads (q|k|v, each 512 cols) and the
w_proj rows for its heads (512 rows).  Per-core output is a partial sum
over head groups; the host adds the two partials per batch.

v2 design changes vs v1 (391us -> target ~250us):
  - All matmul operands bf16 (host-cast weights; x host-transposed to
    x^T [E, S] bf16).  Eliminates the 128 PE transposes and their 128
    DVE psum->sbuf copies; bf16 runs at the same 1 cyc/col as f32r but
    without the N>=256 fast-path restriction, so diagonal causal
    skipping is exact (c0 = 128*dshift, uncapped).
  - Wide exp: scores for two consecutive key tiles go into one
    [128, 1024] PSUM tile (2 banks, one matmul per bank), and one ACT
    exp covers both -> half the ACT per-instruction overhead, which was
    ~35% of ACT busy time.
  - Sums rows ride in the y^T evacuation: av output py[0:65] (64 dims +
    1 sums row) is copied to SBUF in one DVE copy per head; the DMA
    denominator-broadcast bounce reads the row straight out of that
    tile (no separate [1,512] copies).
  - All DMAs issue from nc.sync (SP) or nc.gpsimd, never nc.scalar:
    v1 spent ~92us of ACT sequencer time issuing DMAs, starving exp.

Per-core phases (strips of 512 queries), software-pipelined as in v1:
phase-1 (QKV) of strip s+1 and phase-3/normalize of strip s-1 are
drip-fed between the attention groups of strip s.

PSUM budget (8 banks): ph1 x2 (qkv/v/proj), ps x2 of [128,1024] wide
scores (=4 banks), py x2 ([65,512] exp@V+sums accumulators).

No softmax max-subtraction: scores ~N(0,1), measured |s| <= 8.4; exp is
fp32/bf16-safe.
"""

import os
from contextlib import ExitStack

import numpy as np
import ml_dtypes

import concourse.bass as bass
import concourse.bacc as bacc
import concourse.mybir as mybir
import concourse.tile as tile
from concourse.bass_utils import run_bass_kernel_spmd

F32 = mybir.dt.float32
BF16 = mybir.dt.bfloat16
EXP = mybir.ActivationFunctionType.Exp

S = 2048          # sequence length
E = 1024          # embedding
D = 64            # head dim
HL = 8            # heads per core
NP = 4            # head pairs per core
EC = 8            # E / 128 chunks
NSTRIP = 4        # query strips of 512
TPS = 4           # 128-token tiles per strip
NT = 16           # 128-key tiles total


def emit_kernel(ctx, tc, out, xT, w_qkv, w_proj):
    nc = tc.nc

    const = ctx.enter_context(tc.tile_pool(name="const", bufs=1))
    wpool = ctx.enter_context(tc.tile_pool(name="weights", bufs=1))
    kv = ctx.enter_context(tc.tile_pool(name="kv", bufs=1))
    work = ctx.enter_context(tc.tile_pool(name="work", bufs=1))
    psum = ctx.enter_context(tc.tile_pool(name="psum", bufs=1, space="PSUM"))

    # ---- constants ----
    ones_row8 = const.tile([128, 8], BF16, name="ones_row8")
    nc.gpsimd.memset(ones_row8[:], 1.0)
    # DRAM bounce rows for the softmax-sums broadcast (2 per pair-strip)
    rbounce = nc.dram_tensor("rbounce", [2 * NP * NSTRIP, 512], F32).ap()

    # ---- resident weights (DRAM already bf16, host-cast) ----
    wqk = []
    for e in range(EC):
        t = wpool.tile([128, 1024], BF16, name=f"wqk{e}", tag=f"wqk{e}")
        nc.sync.dma_start(out=t[:], in_=w_qkv[e * 128:(e + 1) * 128, 0:1024])
        wqk.append(t)
    wv = []
    for e in range(EC):
        t = wpool.tile([128, 512], BF16, name=f"wv{e}", tag=f"wv{e}")
        nc.sync.dma_start(out=t[:], in_=w_qkv[e * 128:(e + 1) * 128, 1024:1536])
        wv.append(t)
    wpj = []
    for f in range(NP):
        t = wpool.tile([128, 1024], BF16, name=f"wpj{f}", tag=f"wpj{f}")
        nc.sync.dma_start(out=t[:], in_=w_proj[f * 128:(f + 1) * 128, :])
        wpj.append(t)

    # ---- persistent K^T (pair-stacked) and V||ones (8 heads x 65) ----
    kT = [kv.tile([128, S], BF16, name=f"kT{p}", tag=f"kT{p}")
          for p in range(NP)]
    vaug = [kv.tile([128, 520], BF16, name=f"vaug_{t}", tag=f"vaug_{t}")
            for t in range(NT)]
    # resident x^T, all strips (bf16, 4KB/partition/chunk)
    xTs = [kv.tile([128, S], BF16, name=f"xT{e}", tag=f"xT{e}")
           for e in range(EC)]

    state = {}

    def load_xT():
        for e in range(EC):
            nc.sync.dma_start(out=xTs[e][:],
                              in_=xT[e * 128:(e + 1) * 128, :])

    def copy(use_act, out_ap, in_ap):
        if use_act:
            nc.scalar.copy(out_ap, in_ap)
        else:
            nc.vector.tensor_copy(out_ap, in_ap)

    def qk_chunk(s, p, which, half, use_act=False):
        """Half of the Q^T (or K^T) accumulation for pair p of strip s.
        s >= NSTRIP means strip s - NSTRIP of the NEXT repeat (junction)."""
        if ("qT", s) not in state:
            state[("qT", s)] = [
                work.tile([128, 512], BF16, name=f"qT{p}_{s}",
                          tag=f"qT{p}", bufs=2)
                for p in range(NP)]
        qT = state[("qT", s)]
        co = (0 if which == "q" else 512) + p * 128
        if half == 0:
            pqk = psum.tile([128, 512], F32, name=f"p{which}_{s}_{p}",
                            tag="ph1", bufs=2)
            state[("pqk", s, p, which)] = pqk
        else:
            pqk = state.pop(("pqk", s, p, which))
        for e in range(4 * half, 4 * half + 4):
            nc.tensor.matmul(pqk[:], wqk[e][:, co:co + 128],
                             xTs[e][:, (s % NSTRIP) * 512:(s % NSTRIP + 1) * 512],
                             start=(e == 0), stop=(e == EC - 1))
        if half == 1:
            if which == "q":
                copy(use_act, qT[p][:], pqk[:])
            else:
                copy(use_act, kT[p][:, (s % NSTRIP) * 512:(s % NSTRIP + 1) * 512],
                     pqk[:])

    def v_chunk(s, tt, half, use_act=False):
        """Half of the V||ones accumulation for token tile tt of strip s."""
        t0 = ((s % NSTRIP) * TPS + tt) * 128
        if half == 0:
            pv = psum.tile([128, 512], F32, name=f"pv_{s}_{tt}", tag="ph1",
                           bufs=2)
            state[("pv", s, tt)] = pv
        else:
            pv = state.pop(("pv", s, tt))
        for e in range(4 * half, 4 * half + 4):
            nc.tensor.matmul(pv[:], xTs[e][:, t0:t0 + 128], wv[e][:],
                             start=(e == 0), stop=(e == EC - 1))
        if half == 1:
            # scatter V into the augmented [head*65 .. head*65+64] slots and
            # fill the ones columns, both as single strided copies
            va = vaug[(s % NSTRIP) * TPS + tt]
            va3 = va.rearrange("p (h c) -> p h c", c=65)
            copy(use_act, va3[:, :, 0:64],
                 pv[:].rearrange("p (h c) -> p h c", c=64))
            nc.vector.tensor_copy(va3[:, :, 64:65],
                                  ones_row8[:].rearrange("p (h c) -> p h c", c=1))

    def phase1_units(s, act_frac=0.0, only=None):
        """Phase-1 units for (possibly virtual, s>=4 = next iter) strip s.
        act_frac routes that fraction of the PSUM->SBUF copies to ACT.
        only='q' emits just the Q^T units; only='kv' the K^T and V units
        (Q^T writes fresh tiles, so the next repeat's Q units can drip into
        phase2(3) without write-after-read hazards on kT/vaug)."""
        us = []
        ci = [0]

        def pick():
            ci[0] += 1
            return int(ci[0] * act_frac) != int((ci[0] - 1) * act_frac)

        for p in range(NP):
            for which in ("q", "k"):
                if only == "q" and which != "q":
                    continue
                if only == "kv" and which == "q":
                    continue
                for half in range(2):
                    us.append(lambda s=s, p=p, w=which, h=half,
                              a=(pick() if half else False):
                              qk_chunk(s, p, w, h, a))
        if only != "q":
            for tt in range(TPS):
                for half in range(2):
                    us.append(lambda s=s, tt=tt, h=half,
                              a=(pick() if half else False):
                              v_chunk(s, tt, h, a))
        return us

    def norm_unit(s, p):
        """Deferred softmax normalization for pair p of strip s."""
        def norm():
            yu, recb = state.pop(("norm", s, p))
            yT = state[("yT", s)]
            nc.vector.reciprocal(recb[:], recb[:])
            nc.vector.tensor_mul(yT[p][:], yu[:], recb[:])
        return norm

    def p3_units(s, act_frac=0.0):
        """Projection for strip s as units (one per output tile)."""
        def proj(tt, eo, use_act):
            yT = state[("yT", s)]
            po = psum.tile([128, 512], F32, name=f"po_{s}_{tt}_{eo}",
                           tag="ph1", bufs=2)
            for p in range(NP):
                nc.tensor.matmul(
                    po[:], yT[p][:, tt * 128:(tt + 1) * 128],
                    wpj[p][:, eo * 512:(eo + 1) * 512],
                    start=(p == 0), stop=(p == NP - 1))
            osb = work.tile([128, 512], F32, name=f"osb_{s}_{tt}_{eo}",
                            tag="osb", bufs=2)
            copy(use_act, osb[:], po[:])
            r0 = (s * TPS + tt) * 128
            nc.sync.dma_start(
                out=out[r0:r0 + 128, eo * 512:(eo + 1) * 512], in_=osb[:])
        return [lambda tt=tt, eo=eo, a=(act_frac > 0 and (tt * 2 + eo) % 2 == 1):
                proj(tt, eo, a)
                for tt in range(TPS) for eo in range(2)]

    def phase2(s, units, pair_hook=None):
        """Attention for strip s; `units` drip-fed between groups.  Each
        pair's deferred normalization is late-appended as a unit once the
        NEXT pair's tail has been emitted (its recb DMA has landed by then).
        pair_hook(p) is called right after pair p's tail (used to spread the
        next repeat's x^T prefetch DMAs across strip 3)."""
        qT = state[("qT", s)]
        state[("yT", s)] = [
            work.tile([128, 512], BF16, name=f"yT{p}_{s}", tag=f"yT{p}",
                      bufs=2)
            for p in range(NP)]
        ntile = 4 * s + 4
        ngrp = ntile // 2
        units = list(units)
        nslots = NP * ngrp
        pulled = 0
        slot = 0

        def pull():
            nonlocal pulled, slot
            slot += 1
            while pulled < len(units) and pulled < len(units) * slot / nslots:
                units[pulled]()
                pulled += 1

        for p in range(NP):
            py_a = psum.tile([65, 512], F32, name=f"pya_{s}_{p}", tag="py",
                             bufs=2)
            py_b = psum.tile([65, 512], F32, name=f"pyb_{s}_{p}", tag="py",
                             bufs=2)

            def scores_exp(g):
                # two key tiles -> one [128,1024] psum (2 banks) per head;
                # exp over the valid span(s); causal mask via affine_select
                # on only the 128-col partial band of diagonal tiles.
                ts_ = (2 * g, 2 * g + 1)
                c0s = [max(0, 128 * (t - 4 * s)) for t in ts_]
                ps_a = psum.tile([128, 1024], F32, name=f"psa_{s}_{p}_{g}",
                                 tag="ps", bufs=2)
                ps_b = psum.tile([128, 1024], F32, name=f"psb_{s}_{p}_{g}",
                                 tag="ps", bufs=2)
                for j, (t, c0) in enumerate(zip(ts_, c0s)):
                    ksl = kT[p][:, t * 128:(t + 1) * 128]
                    lo = j * 512 + c0
                    hi = (j + 1) * 512
                    nc.tensor.matmul(ps_a[:, lo:hi], ksl[0:64, :],
                                     qT[p][0:64, c0:], start=True, stop=True)
                    nc.tensor.matmul(ps_b[:, lo:hi], ksl[64:128, :],
                                     qT[p][64:128, c0:],
                                     start=True, stop=True,
                                     tile_position=(64, 0))
                es_a = work.tile([128, 1024], BF16, name=f"esa_{s}_{p}_{g}",
                                 tag="es", bufs=6)
                es_b = work.tile([128, 1024], BF16, name=f"esb_{s}_{p}_{g}",
                                 tag="es", bufs=6)
                for est, pst in ((es_a, ps_a), (es_b, ps_b)):
                    if c0s[0] == 0 and c0s[1] == 0:
                        nc.scalar.activation(est[:], pst[:], EXP, scale=0.125)
                    else:
                        for j, c0 in enumerate(c0s):
                            lo = j * 512 + c0
                            hi = (j + 1) * 512
                            nc.scalar.activation(est[:, lo:hi],
                                                 pst[:, lo:hi], EXP,
                                                 scale=0.125)
                    # mask the 128-col partial band of each diagonal tile
                    for j, (t, c0) in enumerate(zip(ts_, c0s)):
                        if t - 4 * s >= 0:
                            lo = j * 512 + c0
                            nc.gpsimd.affine_select(
                                out=est[:, lo:lo + 128],
                                in_=est[:, lo:lo + 128],
                                compare_op=mybir.AluOpType.is_ge, fill=0.0,
                                base=0, channel_multiplier=-1,
                                pattern=[[1, 128]])
                return es_a, es_b, c0s

            def av_group(g, es_a, es_b, c0s):
                st = (g == 0)
                sp = (g == ngrp - 1)
                for j, c0 in enumerate(c0s):
                    t = 2 * g + j
                    vA = vaug[t][:, (2 * p) * 65:(2 * p) * 65 + 65]
                    vB = vaug[t][:, (2 * p + 1) * 65:(2 * p + 1) * 65 + 65]
                    lo = j * 512 + c0
                    hi = (j + 1) * 512
                    nc.tensor.matmul(py_a[:, c0:], vA, es_a[:, lo:hi],
                                     start=(st and j == 0),
                                     stop=(sp and j == 1))
                    nc.tensor.matmul(py_b[:, c0:], vB, es_b[:, lo:hi],
                                     start=(st and j == 0),
                                     stop=(sp and j == 1))

            # software pipeline: issue scores(g+1) before exp@V(g) so the
            # PE never waits on ACT's exp; drip filler units in per slot.
            prev = scores_exp(0)
            for g in range(1, ngrp):
                cur = scores_exp(g)
                av_group(g - 1, *prev)
                pull()
                prev = cur
            av_group(ngrp - 1, *prev)
            pull()
            del prev

            # pair tail: unnormalized y^T and the sums rows off PSUM (frees
            # the py banks), bounce the sums rows through DRAM to broadcast
            # across partitions, defer reciprocal+multiply to a norm unit
            # (DMA round-trip has landed by then).
            ri = 2 * (s * NP + p)
            yu = work.tile([128, 512], F32, name=f"yu_{s}_{p}",
                           tag=f"yu{p}", bufs=1)
            nc.vector.tensor_copy(yu[0:64, :], py_a[0:64, :])
            nc.vector.tensor_copy(yu[64:128, :], py_b[0:64, :])
            srab = work.tile([1, 1024], F32, name=f"srab_{s}_{p}",
                             tag="srab", bufs=2)
            nc.vector.tensor_copy(srab[:, 0:512], py_a[64:65, :])
            nc.vector.tensor_copy(srab[:, 512:1024], py_b[64:65, :])
            nc.sync.dma_start(
                out=rbounce[ri:ri + 2, :].rearrange("a b -> (a b)").unsqueeze(0),
                in_=srab[:])
            recb = work.tile([128, 512], F32, name=f"recb_{s}_{p}",
                             tag="recb", bufs=3)
            nc.sync.dma_start(
                out=recb[0:64, :],
                in_=rbounce[ri:ri + 1, :].broadcast_to((64, 512)))
            nc.sync.dma_start(
                out=recb[64:128, :],
                in_=rbounce[ri + 1:ri + 2, :].broadcast_to((64, 512)))
            state[("norm", s, p)] = (yu, recb)
            if pair_hook is not None:
                pair_hook(p)
            if p >= 1:
                units.append(norm_unit(s, p - 1))
        while pulled < len(units):
            units[pulled]()
            pulled += 1

    ACT_FRACS = {1: 1.0, 2: 0.5, 3: 0.0}

    def prologue():
        """One-time head: load x^T and run phase 1 of strip 0."""
        load_xT()
        for u in phase1_units(0, act_frac=0.5):
            u()

    def strips_body():
        for s in range(NSTRIP):
            units = []
            if s >= 1:
                units.append(norm_unit(s - 1, 3))
                units.extend(phase1_units(s + 1, act_frac=ACT_FRACS[s + 1])
                             if s + 1 < NSTRIP else [])
                units.extend(p3_units(s - 1))
            else:
                units.extend(phase1_units(1, act_frac=ACT_FRACS[1]))
            if s == NSTRIP - 1:
                # prefetch next repeat's x^T across phase2(3)'s early pair
                # tails (phase1(3), the last reader of the old x^T, ran
                # during phase2(2)), and drip the next repeat's Q^T units in
                # as extra fillers (strip 3 is ACT-bound, the PE has slack;
                # Q^T writes fresh tiles so there is no WAR hazard with this
                # strip's attention reads, unlike K^T/V)
                def prefetch(p):
                    if p < 2:
                        for e in range(4 * p, 4 * p + 4):
                            nc.sync.dma_start(
                                out=xTs[e][:],
                                in_=xT[e * 128:(e + 1) * 128, :])
                    if p == 1:
                        units.extend(phase1_units(NSTRIP, only="q"))
                phase2(s, units, pair_hook=prefetch)
            else:
                phase2(s, units)

    def junction():
        """Tail of this repeat (strip-3 norm + projection) overlapped with
        the head of the next (phase 1 of strip 0; x^T already prefetched)."""
        norm_unit(NSTRIP - 1, 3)()
        p3u = p3_units(NSTRIP - 1, act_frac=0.5)
        p1u = phase1_units(NSTRIP, act_frac=0.5, only="kv")
        # front-load phase-1 units so the PE has ~5us of work queued before
        # the first projection (which waits on the recb DMA round trip +
        # norm), then round-robin 2 phase-1 units per projection unit
        ip1 = 0
        for _ in range(6):
            p1u[ip1]()
            ip1 += 1
        for ip3 in range(len(p3u)):
            p3u[ip3]()
            for _ in range(2):
                if ip1 < len(p1u):
                    p1u[ip1]()
                    ip1 += 1
        while ip1 < len(p1u):
            p1u[ip1]()
            ip1 += 1

    def body():
        strips_body()
        junction()
        # re-point strip-0 state at the tiles the junction just produced so
        # a following body (straight-line unroll) reads them; For_i bodies
        # don't need this (all-engine barrier at the back edge).
        state[("qT", 0)] = state.pop(("qT", NSTRIP))

    repeat = int(os.environ.get("KREPEAT", "1"))
    unroll = int(os.environ.get("KUNROLL", "0"))
    state.clear()
    prologue()
    if unroll > 0:
        for _ in range(unroll):
            body()
    elif repeat > 1:
        with tc.For_i(0, repeat, 1):
            body()
    else:
        body()


_CACHE = {}


def build_nc():
    if "nc" in _CACHE:
        return _CACHE["nc"]
    nc = bacc.Bacc("TRN2", target_bir_lowering=False, debug=False,
                   enable_asserts=False, num_devices=8)
    xT = nc.dram_tensor("xT", [E, S], BF16, kind="ExternalInput").ap()
    w_qkv = nc.dram_tensor("w_qkv", [E, 1536], BF16,
                           kind="ExternalInput").ap()
    w_proj = nc.dram_tensor("w_proj", [512, E], BF16,
                            kind="ExternalInput").ap()
    out = nc.dram_tensor("out", [S, E], F32, kind="ExternalOutput").ap()
    with tile.TileContext(nc) as tc:
        with ExitStack() as ctx:
            emit_kernel(ctx, tc, out, xT, w_qkv, w_proj)
    nc.compile()
    _CACHE["nc"] = nc
    return nc


def make_in_maps(x, w_attn, w_proj):
    x = np.asarray(x, dtype=np.float32)
    w_attn = np.asarray(w_attn, dtype=np.float32)
    w_proj = np.asarray(w_proj, dtype=np.float32)
    bf = ml_dtypes.bfloat16
    in_maps = []
    for c in range(8):
        b, hg = divmod(c, 2)
        lo, hi = hg * 512, (hg + 1) * 512
        wq = w_attn[:, lo:hi]
        wk = w_attn[:, 1024 + lo:1024 + hi]
        wv = w_attn[:, 2048 + lo:2048 + hi]
        wqkv = np.ascontiguousarray(
            np.concatenate([wq, wk, wv], axis=1)).astype(bf)
        wp = np.ascontiguousarray(w_proj[lo:hi, :]).astype(bf)
        xTb = np.ascontiguousarray(x[b].T).astype(bf)
        in_maps.append({
            "xT": xTb,
            "w_qkv": wqkv,
            "w_proj": wp,
        })
    return in_maps


def gather(results):
    parts = [results[c]["out"] for c in range(8)]
    return np.stack([parts[2 * b] + parts[2 * b + 1] for b in range(4)]).astype(
        np.float32)


def kernel(x, w_attn, w_proj):
    nc = build_nc()
    res = run_bass_kernel_spmd(nc, make_in_maps(x, w_attn, w_proj),
                               core_ids=list(range(8)))
    return gather(res.results)


# revision 6
# speedup vs baseline: 1.0491x; 1.0491x over previous
"""Causal self-attention Bass/Tile kernel for 8 Trainium2 NeuronCores.

Problem (hardcoded): x (4, 2048, 1024) f32, w_attn (1024, 3072), w_proj
(1024, 1024).  H=16 heads, D=64.  Output: (4, 2048, 1024) f32.

Sharding: core c handles batch b = c // 2 and head-group hg = c % 2
(8 heads each).  Data parallel on B, tensor parallel on heads: each core
gets the w_attn columns for its heads (q|k|v, each 512 cols) and the
w_proj rows for its heads (512 rows).  Per-core output is a partial sum
over head groups; the host adds the two partials per batch.

Design (measured rel err 5.4e-3 vs the f32 reference; bf16 everywhere --
an offline numpy study showed all-bf16 at 5.4e-3 while ANY fp8 stage
blows past the 2e-2 gate):
  - All matmul operands bf16 (host-cast weights; x host-transposed to
    x^T [E, S] bf16).  Eliminates the v1 PE transposes and their 128
    DVE psum->sbuf copies; bf16 runs at the same 1 cyc/col as f32r but
    without the N>=256 fast-path restriction, so diagonal causal
    skipping is exact (c0 = 128*dshift, uncapped).
  - Wide exp: scores for two consecutive key tiles go into one
    [128, 1024] PSUM tile (2 banks, one matmul per bank), and one ACT
    exp covers both -> half the ACT per-instruction overhead, which was
    ~35% of ACT busy time at [128,512] granularity.
  - All DMAs issue from nc.sync (SP), never nc.scalar: v1 spent ~92us
    of ACT sequencer time issuing DMAs, starving exp dispatch.
  - PSUM->SBUF evacuation copies are split between DVE and ACT
    (act_frac per strip) according to where ACT has slack.

Per-core phases (strips of 512 queries), software-pipelined:
phase-1 (QKV) of strip s+1 and phase-3/normalize of strip s-1 are
drip-fed between the attention groups of strip s; each pair's deferred
softmax normalization late-appends once the next pair's tail is out
(its denominator DMA bounce has landed by then).

Repeat structure for marginal timing (KREPEAT): tc.For_i inserts an
all-engine barrier at every back edge, so the body is arranged as
[strips 0..3, junction] where the junction overlaps THIS repeat's tail
(strip-3 norm + projection) with the NEXT repeat's head (phase 1 of
strip 0; x^T was prefetched across phase2(3)'s pair tails, and the
next repeat's Q^T units drip into ACT-bound strip 3 -- Q^T writes fresh
tiles so there is no WAR hazard, unlike K^T/V which the current strip
still reads).

PSUM budget (8 banks): ph1 x2 (qkv/v/proj), ps x2 of [128,1024] wide
scores (=4 banks), py x2 ([65,512] exp@V+sums accumulators).

No softmax max-subtraction: scores ~N(0,1), measured |s| <= 8.4; exp is
fp32/bf16-safe.
"""

import os
from contextlib import ExitStack

import numpy as np
import ml_dtypes

import concourse.bass as bass
import concourse.bacc as bacc
import concourse.mybir as mybir
import concourse.tile as tile
from concourse.bass_utils import run_bass_kernel_spmd

F32 = mybir.dt.float32
BF16 = mybir.dt.bfloat16
EXP = mybir.ActivationFunctionType.Exp

S = 2048          # sequence length
E = 1024          # embedding
D = 64            # head dim
HL = 8            # heads per core
NP = 4            # head pairs per core
EC = 8            # E / 128 chunks
NSTRIP = 4        # query strips of 512
TPS = 4           # 128-token tiles per strip
NT = 16           # 128-key tiles total


def emit_kernel(ctx, tc, out, xT, w_qkv, w_proj):
    nc = tc.nc

    const = ctx.enter_context(tc.tile_pool(name="const", bufs=1))
    wpool = ctx.enter_context(tc.tile_pool(name="weights", bufs=1))
    kv = ctx.enter_context(tc.tile_pool(name="kv", bufs=1))
    work = ctx.enter_context(tc.tile_pool(name="work", bufs=1))
    psum = ctx.enter_context(tc.tile_pool(name="psum", bufs=1, space="PSUM"))

    # ---- constants ----
    ones_row8 = const.tile([128, 8], BF16, name="ones_row8")
    nc.gpsimd.memset(ones_row8[:], 1.0)
    # DRAM bounce rows for the softmax-sums broadcast (2 per pair-strip)
    rbounce = nc.dram_tensor("rbounce", [2 * NP * NSTRIP, 512], F32).ap()

    # ---- resident weights (DRAM already bf16, host-cast) ----
    wqk = []
    for e in range(EC):
        t = wpool.tile([128, 1024], BF16, name=f"wqk{e}", tag=f"wqk{e}")
        nc.sync.dma_start(out=t[:], in_=w_qkv[e * 128:(e + 1) * 128, 0:1024])
        wqk.append(t)
    wv = []
    for e in range(EC):
        t = wpool.tile([128, 512], BF16, name=f"wv{e}", tag=f"wv{e}")
        nc.sync.dma_start(out=t[:], in_=w_qkv[e * 128:(e + 1) * 128, 1024:1536])
        wv.append(t)
    wpj = []
    for f in range(NP):
        t = wpool.tile([128, 1024], BF16, name=f"wpj{f}", tag=f"wpj{f}")
        nc.sync.dma_start(out=t[:], in_=w_proj[f * 128:(f + 1) * 128, :])
        wpj.append(t)

    # ---- persistent K^T (pair-stacked) and V||ones (8 heads x 65) ----
    kT = [kv.tile([128, S], BF16, name=f"kT{p}", tag=f"kT{p}")
          for p in range(NP)]
    vaug = [kv.tile([128, 520], BF16, name=f"vaug_{t}", tag=f"vaug_{t}")
            for t in range(NT)]
    # resident x^T, all strips (bf16, 4KB/partition/chunk)
    xTs = [kv.tile([128, S], BF16, name=f"xT{e}", tag=f"xT{e}")
           for e in range(EC)]

    state = {}

    def load_xT():
        for e in range(EC):
            nc.sync.dma_start(out=xTs[e][:],
                              in_=xT[e * 128:(e + 1) * 128, :])

    def copy(use_act, out_ap, in_ap):
        if use_act:
            nc.scalar.copy(out_ap, in_ap)
        else:
            nc.vector.tensor_copy(out_ap, in_ap)

    def qk_chunk(s, p, which, half, use_act=False):
        """Half of the Q^T (or K^T) accumulation for pair p of strip s.
        s >= NSTRIP means strip s - NSTRIP of the NEXT repeat (junction)."""
        if ("qT", s) not in state:
            state[("qT", s)] = [
                work.tile([128, 512], BF16, name=f"qT{p}_{s}",
                          tag=f"qT{p}", bufs=2)
                for p in range(NP)]
        qT = state[("qT", s)]
        co = (0 if which == "q" else 512) + p * 128
        if half == 0:
            pqk = psum.tile([128, 512], F32, name=f"p{which}_{s}_{p}",
                            tag="ph1", bufs=2)
            state[("pqk", s, p, which)] = pqk
        else:
            pqk = state.pop(("pqk", s, p, which))
        for e in range(4 * half, 4 * half + 4):
            nc.tensor.matmul(pqk[:], wqk[e][:, co:co + 128],
                             xTs[e][:, (s % NSTRIP) * 512:(s % NSTRIP + 1) * 512],
                             start=(e == 0), stop=(e == EC - 1))
        if half == 1:
            if which == "q":
                copy(use_act, qT[p][:], pqk[:])
            else:
                copy(use_act, kT[p][:, (s % NSTRIP) * 512:(s % NSTRIP + 1) * 512],
                     pqk[:])

    def v_chunk(s, tt, half, use_act=False):
        """Half of the V||ones accumulation for token tile tt of strip s."""
        t0 = ((s % NSTRIP) * TPS + tt) * 128
        if half == 0:
            pv = psum.tile([128, 512], F32, name=f"pv_{s}_{tt}", tag="ph1",
                           bufs=2)
            state[("pv", s, tt)] = pv
        else:
            pv = state.pop(("pv", s, tt))
        for e in range(4 * half, 4 * half + 4):
            nc.tensor.matmul(pv[:], xTs[e][:, t0:t0 + 128], wv[e][:],
                             start=(e == 0), stop=(e == EC - 1))
        if half == 1:
            # scatter V into the augmented [head*65 .. head*65+64] slots and
            # fill the ones columns, both as single strided copies
            va = vaug[(s % NSTRIP) * TPS + tt]
            va3 = va.rearrange("p (h c) -> p h c", c=65)
            copy(use_act, va3[:, :, 0:64],
                 pv[:].rearrange("p (h c) -> p h c", c=64))
            nc.vector.tensor_copy(va3[:, :, 64:65],
                                  ones_row8[:].rearrange("p (h c) -> p h c", c=1))

    def phase1_units(s, act_frac=0.0, only=None):
        """Phase-1 units for (possibly virtual, s>=4 = next iter) strip s.
        act_frac routes that fraction of the PSUM->SBUF copies to ACT.
        only='q' emits just the Q^T units; only='kv' the K^T and V units
        (Q^T writes fresh tiles, so the next repeat's Q units can drip into
        phase2(3) without write-after-read hazards on kT/vaug)."""
        us = []
        ci = [0]

        def pick():
            ci[0] += 1
            return int(ci[0] * act_frac) != int((ci[0] - 1) * act_frac)

        for p in range(NP):
            for which in ("q", "k"):
                if only == "q" and which != "q":
                    continue
                if only == "kv" and which == "q":
                    continue
                for half in range(2):
                    us.append(lambda s=s, p=p, w=which, h=half,
                              a=(pick() if half else False):
                              qk_chunk(s, p, w, h, a))
        if only != "q":
            for tt in range(TPS):
                for half in range(2):
                    us.append(lambda s=s, tt=tt, h=half,
                              a=(pick() if half else False):
                              v_chunk(s, tt, h, a))
        return us

    def norm_unit(s, p):
        """Deferred softmax normalization for pair p of strip s."""
        def norm():
            yu, recb = state.pop(("norm", s, p))
            yT = state[("yT", s)]
            nc.vector.reciprocal(recb[:], recb[:])
            nc.vector.tensor_mul(yT[p][:], yu[:], recb[:])
        return norm

    def p3_units(s, act_frac=0.0):
        """Projection for strip s as units (one per output tile)."""
        def proj(tt, eo, use_act):
            yT = state[("yT", s)]
            po = psum.tile([128, 512], F32, name=f"po_{s}_{tt}_{eo}",
                           tag="ph1", bufs=2)
            for p in range(NP):
                nc.tensor.matmul(
                    po[:], yT[p][:, tt * 128:(tt + 1) * 128],
                    wpj[p][:, eo * 512:(eo + 1) * 512],
                    start=(p == 0), stop=(p == NP - 1))
            osb = work.tile([128, 512], F32, name=f"osb_{s}_{tt}_{eo}",
                            tag="osb", bufs=2)
            copy(use_act, osb[:], po[:])
            r0 = (s * TPS + tt) * 128
            nc.sync.dma_start(
                out=out[r0:r0 + 128, eo * 512:(eo + 1) * 512], in_=osb[:])
        return [lambda tt=tt, eo=eo, a=(act_frac > 0 and (tt * 2 + eo) % 2 == 1):
                proj(tt, eo, a)
                for tt in range(TPS) for eo in range(2)]

    def phase2(s, units, pair_hook=None):
        """Attention for strip s; `units` drip-fed between groups.  Each
        pair's deferred normalization is late-appended as a unit once the
        NEXT pair's tail has been emitted (its recb DMA has landed by then).
        pair_hook(p) is called right after pair p's tail (used to spread the
        next repeat's x^T prefetch DMAs across strip 3)."""
        qT = state[("qT", s)]
        state[("yT", s)] = [
            work.tile([128, 512], BF16, name=f"yT{p}_{s}", tag=f"yT{p}",
                      bufs=2)
            for p in range(NP)]
        ntile = 4 * s + 4
        ngrp = ntile // 2
        units = list(units)
        nslots = NP * ngrp
        pulled = 0
        slot = 0

        def pull():
            nonlocal pulled, slot
            slot += 1
            while pulled < len(units) and pulled < len(units) * slot / nslots:
                units[pulled]()
                pulled += 1

        for p in range(NP):
            py_a = psum.tile([65, 512], F32, name=f"pya_{s}_{p}", tag="py",
                             bufs=2)
            py_b = psum.tile([65, 512], F32, name=f"pyb_{s}_{p}", tag="py",
                             bufs=2)

            def scores_exp(g):
                # two key tiles -> one [128,1024] psum (2 banks) per head;
                # exp over the valid span(s); causal mask via affine_select
                # on only the 128-col partial band of diagonal tiles.
                ts_ = (2 * g, 2 * g + 1)
                c0s = [max(0, 128 * (t - 4 * s)) for t in ts_]
                ps_a = psum.tile([128, 1024], F32, name=f"psa_{s}_{p}_{g}",
                                 tag="ps", bufs=2)
                ps_b = psum.tile([128, 1024], F32, name=f"psb_{s}_{p}_{g}",
                                 tag="ps", bufs=2)
                for j, (t, c0) in enumerate(zip(ts_, c0s)):
                    ksl = kT[p][:, t * 128:(t + 1) * 128]
                    lo = j * 512 + c0
                    hi = (j + 1) * 512
                    nc.tensor.matmul(ps_a[:, lo:hi], ksl[0:64, :],
                                     qT[p][0:64, c0:], start=True, stop=True)
                    nc.tensor.matmul(ps_b[:, lo:hi], ksl[64:128, :],
                                     qT[p][64:128, c0:],
                                     start=True, stop=True,
                                     tile_position=(64, 0))
                es_a = work.tile([128, 1024], BF16, name=f"esa_{s}_{p}_{g}",
                                 tag="es", bufs=6)
                es_b = work.tile([128, 1024], BF16, name=f"esb_{s}_{p}_{g}",
                                 tag="es", bufs=6)
                for est, pst in ((es_a, ps_a), (es_b, ps_b)):
                    if c0s[0] == 0 and c0s[1] == 0:
                        nc.scalar.activation(est[:], pst[:], EXP, scale=0.125)
                    else:
                        for j, c0 in enumerate(c0s):
                            lo = j * 512 + c0
                            hi = (j + 1) * 512
                            nc.scalar.activation(est[:, lo:hi],
                                                 pst[:, lo:hi], EXP,
                                                 scale=0.125)
                    # mask the 128-col partial band of each diagonal tile
                    for j, (t, c0) in enumerate(zip(ts_, c0s)):
                        if t - 4 * s >= 0:
                            lo = j * 512 + c0
                            nc.gpsimd.affine_select(
                                out=est[:, lo:lo + 128],
                                in_=est[:, lo:lo + 128],
                                compare_op=mybir.AluOpType.is_ge, fill=0.0,
                                base=0, channel_multiplier=-1,
                                pattern=[[1, 128]])
                return es_a, es_b, c0s

            def av_group(g, es_a, es_b, c0s):
                st = (g == 0)
                sp = (g == ngrp - 1)
                for j, c0 in enumerate(c0s):
                    t = 2 * g + j
                    vA = vaug[t][:, (2 * p) * 65:(2 * p) * 65 + 65]
                    vB = vaug[t][:, (2 * p + 1) * 65:(2 * p + 1) * 65 + 65]
                    lo = j * 512 + c0
                    hi = (j + 1) * 512
                    nc.tensor.matmul(py_a[:, c0:], vA, es_a[:, lo:hi],
                                     start=(st and j == 0),
                                     stop=(sp and j == 1))
                    nc.tensor.matmul(py_b[:, c0:], vB, es_b[:, lo:hi],
                                     start=(st and j == 0),
                                     stop=(sp and j == 1))

            # software pipeline: issue scores(g+1) before exp@V(g) so the
            # PE never waits on ACT's exp; drip filler units in per slot.
            prev = scores_exp(0)
            for g in range(1, ngrp):
                cur = scores_exp(g)
                av_group(g - 1, *prev)
                pull()
                prev = cur
            av_group(ngrp - 1, *prev)
            pull()
            del prev

            # pair tail: unnormalized y^T and the sums rows off PSUM (frees
            # the py banks), bounce the sums rows through DRAM to broadcast
            # across partitions, defer reciprocal+multiply to a norm unit
            # (DMA round-trip has landed by then).
            ri = 2 * (s * NP + p)
            yu = work.tile([128, 512], F32, name=f"yu_{s}_{p}",
                           tag=f"yu{p}", bufs=1)
            nc.vector.tensor_copy(yu[0:64, :], py_a[0:64, :])
            nc.vector.tensor_copy(yu[64:128, :], py_b[0:64, :])
            srab = work.tile([1, 1024], F32, name=f"srab_{s}_{p}",
                             tag="srab", bufs=2)
            nc.vector.tensor_copy(srab[:, 0:512], py_a[64:65, :])
            nc.vector.tensor_copy(srab[:, 512:1024], py_b[64:65, :])
            nc.sync.dma_start(
                out=rbounce[ri:ri + 2, :].rearrange("a b -> (a b)").unsqueeze(0),
                in_=srab[:])
            recb = work.tile([128, 512], F32, name=f"recb_{s}_{p}",
                             tag="recb", bufs=3)
            nc.sync.dma_start(
                out=recb[0:64, :],
                in_=rbounce[ri:ri + 1, :].broadcast_to((64, 512)))
            nc.sync.dma_start(
                out=recb[64:128, :],
                in_=rbounce[ri + 1:ri + 2, :].broadcast_to((64, 512)))
            state[("norm", s, p)] = (yu, recb)
            if pair_hook is not None:
                pair_hook(p)
            if p >= 1:
                units.append(norm_unit(s, p - 1))
        while pulled < len(units):
            units[pulled]()
            pulled += 1

    ACT_FRACS = {1: 1.0, 2: 0.5, 3: 0.0}

    def prologue():
        """One-time head: load x^T and run phase 1 of strip 0."""
        load_xT()
        for u in phase1_units(0, act_frac=0.5):
            u()

    def strips_body():
        for s in range(NSTRIP):
            units = []
            if s >= 1:
                units.append(norm_unit(s - 1, 3))
                units.extend(phase1_units(s + 1, act_frac=ACT_FRACS[s + 1])
                             if s + 1 < NSTRIP else [])
                units.extend(p3_units(s - 1))
            else:
                units.extend(phase1_units(1, act_frac=ACT_FRACS[1]))
            if s == NSTRIP - 1:
                # prefetch next repeat's x^T across phase2(3)'s early pair
                # tails (phase1(3), the last reader of the old x^T, ran
                # during phase2(2)), and drip the next repeat's Q^T units in
                # as extra fillers (strip 3 is ACT-bound, the PE has slack;
                # Q^T writes fresh tiles so there is no WAR hazard with this
                # strip's attention reads, unlike K^T/V)
                def prefetch(p):
                    if p < 2:
                        for e in range(4 * p, 4 * p + 4):
                            nc.sync.dma_start(
                                out=xTs[e][:],
                                in_=xT[e * 128:(e + 1) * 128, :])
                    if p == 1:
                        units.extend(phase1_units(NSTRIP, only="q"))
                phase2(s, units, pair_hook=prefetch)
            else:
                phase2(s, units)

    def junction():
        """Tail of this repeat (strip-3 norm + projection) overlapped with
        the head of the next (phase 1 of strip 0; x^T already prefetched)."""
        norm_unit(NSTRIP - 1, 3)()
        p3u = p3_units(NSTRIP - 1, act_frac=0.5)
        p1u = phase1_units(NSTRIP, act_frac=0.5, only="kv")
        # front-load phase-1 units so the PE has ~5us of work queued before
        # the first projection (which waits on the recb DMA round trip +
        # norm), then round-robin 2 phase-1 units per projection unit
        ip1 = 0
        for _ in range(6):
            p1u[ip1]()
            ip1 += 1
        for ip3 in range(len(p3u)):
            p3u[ip3]()
            for _ in range(2):
                if ip1 < len(p1u):
                    p1u[ip1]()
                    ip1 += 1
        while ip1 < len(p1u):
            p1u[ip1]()
            ip1 += 1

    def body():
        strips_body()
        junction()
        # re-point strip-0 state at the tiles the junction just produced so
        # a following body (straight-line unroll) reads them; For_i bodies
        # don't need this (all-engine barrier at the back edge).
        state[("qT", 0)] = state.pop(("qT", NSTRIP))

    repeat = int(os.environ.get("KREPEAT", "1"))
    unroll = int(os.environ.get("KUNROLL", "0"))
    state.clear()
    prologue()
    if unroll > 0:
        for _ in range(unroll):
            body()
    elif repeat > 1:
        with tc.For_i(0, repeat, 1):
            body()
    else:
        body()


_CACHE = {}


def build_nc():
    if "nc" in _CACHE:
        return _CACHE["nc"]
    nc = bacc.Bacc("TRN2", target_bir_lowering=False, debug=False,
                   enable_asserts=False, num_devices=8)
    xT = nc.dram_tensor("xT", [E, S], BF16, kind="ExternalInput").ap()
    w_qkv = nc.dram_tensor("w_qkv", [E, 1536], BF16,
                           kind="ExternalInput").ap()
    w_proj = nc.dram_tensor("w_proj", [512, E], BF16,
                            kind="ExternalInput").ap()
    out = nc.dram_tensor("out", [S, E], F32, kind="ExternalOutput").ap()
    with tile.TileContext(nc) as tc:
        with ExitStack() as ctx:
            emit_kernel(ctx, tc, out, xT, w_qkv, w_proj)
    nc.compile()
    _CACHE["nc"] = nc
    return nc


def make_in_maps(x, w_attn, w_proj):
    x = np.asarray(x, dtype=np.float32)
    w_attn = np.asarray(w_attn, dtype=np.float32)
    w_proj = np.asarray(w_proj, dtype=np.float32)
    bf = ml_dtypes.bfloat16
    in_maps = []
    for c in range(8):
        b, hg = divmod(c, 2)
        lo, hi = hg * 512, (hg + 1) * 512
        wq = w_attn[:, lo:hi]
        wk = w_attn[:, 1024 + lo:1024 + hi]
        wv = w_attn[:, 2048 + lo:2048 + hi]
        wqkv = np.ascontiguousarray(
            np.concatenate([wq, wk, wv], axis=1)).astype(bf)
        wp = np.ascontiguousarray(w_proj[lo:hi, :]).astype(bf)
        xTb = np.ascontiguousarray(x[b].T).astype(bf)
        in_maps.append({
            "xT": xTb,
            "w_qkv": wqkv,
            "w_proj": wp,
        })
    return in_maps


def gather(results):
    parts = [results[c]["out"] for c in range(8)]
    return np.stack([parts[2 * b] + parts[2 * b + 1] for b in range(4)]).astype(
        np.float32)


def kernel(x, w_attn, w_proj):
    nc = build_nc()
    res = run_bass_kernel_spmd(nc, make_in_maps(x, w_attn, w_proj),
                               core_ids=list(range(8)))
    return gather(res.results)
